# revision 32
# baseline (speedup 1.0000x reference)
"""Distributed brute-force kNN retrieval (cosine similarity) on 8 Trainium2 cores.

Strategy:
  - Shard the feature bank along N across 8 cores (62500 rows each).
  - Host pre-tiles each shard into fp8, grouped so every DMA is one fully
    contiguous HBM block (128 partitions x up-to-48KB per partition).
    Groups are small at the start (so the first matmuls/scans begin early)
    and at the end (so the serial tail after the last DMA is minimal).
  - Each core computes raw dot products q @ f_shard.T with fp8 matmuls
    (fp32 PSUM accumulation). A chunk pair (2j, 2j+1) lands in one PSUM
    bank ([128, 512] tile, 500 used): queries x chunk 2j on partitions
    0-63, queries x chunk 2j+1 on partitions 64-127 via PE column tiling
    (tile_position=(0,64)).
  - DVE Max8/MaxIndex run DIRECTLY on PSUM (no PSUM->SBUF copy), one
    500-col scan per pair; the odd 125th chunk is a final half block.
  - Candidate vals/idx accumulate in SBUF and drain to HBM progressively
    (on both HWDGE rings) so the final output DMA is tiny.
  - Host maps candidates to global rows, rescores them exactly in fp32
    (normalized cosine, same math as the reference), reduces to top-k and
    gathers the data segments.

Safety margin: top-8 of every 500-col block when only the global top-5
is needed makes the device pass insensitive to fp8 rounding (dot-noise
sigma ~1.4 vs. rank margins of tens of sigma); the exact host rescore
then removes all remaining matmul error.
"""

import os
import sys

import numpy as np

import concourse.bacc as bacc
import concourse.mybir as mybir
from concourse.tile import TileContext
from concourse.bass_utils import run_bass_kernel_spmd


def _ensure_ntff_hook():
    """run_bass_kernel_spmd(trace) under axon imports antenv.axon_hooks,
    which this container image lacks. Provide the shim (profiling works) or
    disable tracing so a stray BASS_TRACE env var cannot crash the run."""
    try:
        import antenv.axon_hooks  # noqa: F401
        return
    except ImportError:
        pass
    try:
        import types
        from trn_agent_boot.trn_boot import _ntff_profile_via_ctypes
        hook = _ntff_profile_via_ctypes("/opt/axon/libaxon_pjrt.so")
        mod = types.ModuleType("antenv.axon_hooks")
        mod.get_axon_ntff_profile_hook = lambda: hook
        mod.set_axon_ntff_profile_hook = lambda h: None
        sys.modules["antenv.axon_hooks"] = mod
        import antenv
        antenv.axon_hooks = mod
    except Exception:
        os.environ["BASS_NEVER_TRACE"] = "1"

# Problem geometry (hardcoded per spec).
B = 64             # queries
D = 768            # feature dim
N = 500000         # feature rows
NCORES = 8
NSH = N // NCORES  # 62500 rows per core
KC = D // 128      # 6 contraction chunks of 128
CHUNK = 500        # matmul moving free dim; PSUM bank holds 512 fp32
NCHUNKS = NSH // CHUNK   # 125
NPAIRS = 62              # pairs (2j, 2j+1) cover chunks 0..123; chunk 124 alone

# DMA groups (chunk counts); contiguous HBM block per group. Fine-grained
# groups keep the matmul/scan pipeline DMA-paced (no bursty group waits);
# small head groups start compute early; small tail groups shrink the
# serial tail after the last byte lands.
GROUPS = [2, 2, 4] + [8] * 14 + [2, 2, 1]
assert sum(GROUPS) == NCHUNKS
CHUNK_ORDER = list(range(NCHUNKS))
CHUNK_POS = {c: p for p, c in enumerate(CHUNK_ORDER)}
GW = max(GROUPS)
PERCH = KC * CHUNK  # bytes per partition per chunk (fp8) = 3000

NBLOCKS = NPAIRS + 1  # one 500-col scan per pair + the lone chunk 124
TOPB = 8
OUTW = NBLOCKS * TOPB  # 504
# Progressive output drains after these block indices (prefix col ranges).
DRAINS = [(30, 0, 31 * TOPB), (55, 31 * TOPB, 56 * TOPB), (NBLOCKS - 1, 56 * TOPB, OUTW)]

_COMPILED = None
LAST_RESULTS = None  # test harness introspection


def _build():
    nc = bacc.Bacc("TRN2", target_bir_lowering=False, debug=False)
    qT = nc.declare_dram_parameter("qT", [128, KC * B], mybir.dt.float8e4, isOutput=False)
    fT = nc.declare_dram_parameter("fT", [NSH * D], mybir.dt.float8e4, isOutput=False)
    out_vals = nc.declare_dram_parameter(
        "vals", [128, OUTW], mybir.dt.float32, isOutput=True
    )
    out_idx = nc.declare_dram_parameter(
        "idx", [128, OUTW], mybir.dt.uint16, isOutput=True
    )

    with TileContext(nc) as tc:
        with (
            tc.tile_pool(name="qpool", bufs=1) as qpool,
            tc.tile_pool(name="fpool", bufs=8) as fpool,
            tc.tile_pool(name="outpool", bufs=1) as outpool,
            tc.tile_pool(name="psum", bufs=8, space="PSUM") as psump,
        ):
            q_sb = qpool.tile([128, KC, B], mybir.dt.float8e4)
            nc.scalar.dma_start(
                out=q_sb[:], in_=qT.ap().rearrange("p (k m) -> p k m", k=KC)
            )

            vals_st = outpool.tile([128, OUTW], mybir.dt.float32)
            idx_st = outpool.tile([128, OUTW], mybir.dt.uint16)

            chunk_views = {}   # chunk id -> SBUF AP [128, KC, CHUNK]
            loaded = [0]
            goff = [0]         # flat fp8 offset of next group
            gidx = [0]

            def load_until(c):
                pos = CHUNK_POS[c]
                while loaded[0] <= pos:
                    gw = GROUPS[gidx[0]]
                    f_sb = fpool.tile([128, GW * PERCH], mybir.dt.float8e4)
                    sz = gw * PERCH
                    nc.sync.dma_start(
                        out=f_sb[:, :sz],
                        in_=fT.ap()[goff[0] : goff[0] + 128 * sz].rearrange(
                            "(p n) -> p n", p=128
                        ),
                    )
                    for ci in range(gw):
                        chunk_views[CHUNK_ORDER[loaded[0] + ci]] = f_sb[
                            :, ci * PERCH : (ci + 1) * PERCH
                        ].rearrange("p (k n) -> p k n", k=KC)
                    goff[0] += 128 * sz
                    loaded[0] += gw
                    gidx[0] += 1

            def mm_half(ps_cols, chunk, half):
                for k in range(KC):
                    nc.tensor.matmul(
                        ps_cols[half * B : (half + 1) * B, :],
                        lhsT=q_sb[:, k, :],
                        rhs=chunk_views[chunk][:, k, :],
                        start=(k == 0),
                        stop=(k == KC - 1),
                        tile_position=(0, half * B) if half else None,
                    )

            for blk in range(NBLOCKS):
                ps = psump.tile([128, 512], mybir.dt.float32)
                if blk < NPAIRS:
                    load_until(2 * blk)
                    mm_half(ps[:, :CHUNK], 2 * blk, 0)
                    load_until(2 * blk + 1)
                    mm_half(ps[:, :CHUNK], 2 * blk + 1, 1)
                else:  # lone chunk 124: partitions 64-127 scan stale PSUM,
                    # and the host drops those slots (lone block, h==1).
                    load_until(NCHUNKS - 1)
                    mm_half(ps[:, :CHUNK], NCHUNKS - 1, 0)
                scan = ps[:, :CHUNK]
                vslot = vals_st[:, blk * TOPB : (blk + 1) * TOPB]
                nc.vector.max(out=vslot, in_=scan)
                nc.vector.max_index(
                    out=idx_st[:, blk * TOPB : (blk + 1) * TOPB],
                    in_max=vslot,
                    in_values=scan,
                )
                for dblk, c0, c1 in DRAINS:
                    if blk == dblk:
                        # Mid-stream drains must stay OFF the sync ring: the
                        # HWDGE queue is in-order, so a drain gated on DVE
                        # progress would block the feature groups behind it.
                        # Only the final idx drain (no features left) uses
                        # sync, so the two last drains complete in parallel.
                        nc.scalar.dma_start(
                            out=out_vals.ap()[:, c0:c1], in_=vals_st[:, c0:c1]
                        )
                        idx_ring = nc.sync if blk == NBLOCKS - 1 else nc.scalar
                        idx_ring.dma_start(
                            out=out_idx.ap()[:, c0:c1], in_=idx_st[:, c0:c1]
                        )

    nc.compile()
    return nc


def _get_compiled():
    global _COMPILED
    if _COMPILED is None:
        _COMPILED = _build()
    return _COMPILED


def _pretile(f_shard, F8):
    """[62500, 768] fp32 -> flat fp8 buffer in per-group contiguous layout
    following CHUNK_ORDER: group g -> [128 partitions][chunk][KC][500],
    partition-major."""
    f8 = f_shard.astype(F8)
    parts = []
    pos = 0
    for gw in GROUPS:
        ids = CHUNK_ORDER[pos : pos + gw]
        rows = np.concatenate([f8[c * CHUNK : (c + 1) * CHUNK] for c in ids])
        sub = rows.reshape(gw, CHUNK, KC, 128)            # (ci, j, k, p)
        parts.append(np.ascontiguousarray(sub.transpose(3, 0, 2, 1)).reshape(-1))
        pos += gw
    return np.concatenate(parts)


def _candidates(idx_arr, val_arr):
    """Map device outputs (128, 504) to per-query (rows, vals).

    Row q < 64 covers the first chunk of each pair (h=0); row q+64 the
    second (h=1). Block b < 62 is pair b; block 62 is the lone chunk 124
    (valid only for h=0). Returns (B, 2*504); invalid slots get -inf val.
    """
    blk = np.repeat(np.arange(NBLOCKS), TOPB)  # (504,)
    lone = blk == NPAIRS
    rows_out = np.empty((B, 2 * OUTW), dtype=np.int64)
    vals_out = np.empty((B, 2 * OUTW), dtype=np.float64)
    for h in (0, 1):
        i = idx_arr[h * B : (h + 1) * B].astype(np.int64)       # (64, 504)
        v = val_arr[h * B : (h + 1) * B].astype(np.float64)
        feat = np.where(lone, (NCHUNKS - 1) * CHUNK + i, (2 * blk + h) * CHUNK + i)
        if h == 1:  # lone chunk block has no h=1 half
            v = np.where(lone, -np.inf, v)
        rows_out[:, h * OUTW : (h + 1) * OUTW] = feat
        vals_out[:, h * OUTW : (h + 1) * OUTW] = v
    return rows_out, vals_out


def kernel(query_feature, feature, data, k=5, **kwargs):
    global LAST_RESULTS
    q = np.ascontiguousarray(np.asarray(query_feature, dtype=np.float32))
    f = np.asarray(feature, dtype=np.float32)
    data = np.asarray(data)
    k = int(k)
    assert q.shape == (B, D) and f.shape == (N, D)

    nc = _get_compiled()

    F8 = mybir.dt.np(mybir.dt.float8e4)
    # qT[p, k*64+m] = q[m, k*128+p]
    qT = np.ascontiguousarray(
        q.astype(F8).reshape(B, KC, 128).transpose(2, 1, 0)
    ).reshape(128, KC * B)
    in_maps = []
    for i in range(NCORES):
        in_maps.append({"qT": qT, "fT": _pretile(f[i * NSH : (i + 1) * NSH], F8)})

    _ensure_ntff_hook()
    res = run_bass_kernel_spmd(nc, in_maps, core_ids=list(range(NCORES)))
    LAST_RESULTS = res

    all_rows, all_vals = [], []
    for i in range(NCORES):
        rows, vals = _candidates(res.results[i]["idx"], res.results[i]["vals"])
        all_rows.append(i * NSH + rows)
        all_vals.append(vals)
    cand_all = np.concatenate(all_rows, axis=1)  # (B, NCORES*1008)
    vals_all = np.concatenate(all_vals, axis=1)

    # Prefilter by device dot value (fp8 noise sigma ~1.4 on margins ~30
    # sigma): keep the top PREK per query, then rescore those exactly.
    PREK = 96
    pre = np.argpartition(-vals_all, PREK, axis=1)[:, :PREK]
    cand = np.take_along_axis(cand_all, pre, axis=1)  # (B, PREK)

    # Exact fp32 rescore of candidates (same math as the reference).
    qn = q / np.linalg.norm(q, axis=1, keepdims=True)
    fc = f[cand]  # (B, C, D)
    fn = fc / np.linalg.norm(fc, axis=2, keepdims=True)
    sims = np.einsum("bd,bcd->bc", qn, fn)  # fp32

    # Final top-k with jax.lax.top_k tie-breaking (value desc, index asc).
    # Exact fp32 ties inside a block can make Max8/MaxIndex emit duplicate
    # candidates: sort by index, mask duplicate neighbors.
    o = np.argsort(cand, axis=1, kind="stable")
    cand_s = np.take_along_axis(cand, o, axis=1)
    sims_s = np.take_along_axis(sims, o, axis=1)
    dup = np.zeros_like(sims_s, dtype=bool)
    dup[:, 1:] = cand_s[:, 1:] == cand_s[:, :-1]
    sims_s = np.where(dup, -np.inf, sims_s)
    sel = np.argsort(-sims_s, axis=1, kind="stable")[:, :k]
    top_idx = np.take_along_axis(cand_s, sel, axis=1)  # (B, k)

    return data[top_idx]  # (B, k, data_cols), input dtype preserved


# revision 34
# speedup vs baseline: 1.0300x; 1.0300x over previous
"""Distributed brute-force kNN retrieval (cosine similarity) on 8 Trainium2 cores.

Strategy:
  - Shard the feature bank along N across 8 cores (62500 rows each).
  - Host pre-tiles each shard into fp8, grouped so every DMA is one fully
    contiguous HBM block (128 partitions x up-to-48KB per partition).
    Groups are small at the start (so the first matmuls/scans begin early)
    and at the end (so the serial tail after the last DMA is minimal).
  - Each core computes raw dot products q @ f_shard.T with fp8 matmuls
    (fp32 PSUM accumulation). A chunk pair (2j, 2j+1) lands in one PSUM
    bank ([128, 512] tile, 500 used): queries x chunk 2j on partitions
    0-63, queries x chunk 2j+1 on partitions 64-127 via PE column tiling
    (tile_position=(0,64)).
  - DVE Max8/MaxIndex run DIRECTLY on PSUM (no PSUM->SBUF copy), one
    500-col scan per pair; the odd 125th chunk is a final half block.
  - Candidate vals/idx accumulate in SBUF and drain to HBM progressively
    (on both HWDGE rings) so the final output DMA is tiny.
  - Host maps candidates to global rows, rescores them exactly in fp32
    (normalized cosine, same math as the reference), reduces to top-k and
    gathers the data segments.

Safety margin: top-8 of every 500-col block when only the global top-5
is needed makes the device pass insensitive to fp8 rounding (dot-noise
sigma ~1.4 vs. rank margins of tens of sigma); the exact host rescore
then removes all remaining matmul error.
"""

import os
import sys

import numpy as np

import concourse.bacc as bacc
import concourse.mybir as mybir
from concourse.tile import TileContext
from concourse.bass_utils import run_bass_kernel_spmd


def _ensure_ntff_hook():
    """run_bass_kernel_spmd(trace) under axon imports antenv.axon_hooks,
    which this container image lacks. Provide the shim (profiling works) or
    disable tracing so a stray BASS_TRACE env var cannot crash the run."""
    try:
        import antenv.axon_hooks  # noqa: F401
        return
    except ImportError:
        pass
    try:
        import types
        from trn_agent_boot.trn_boot import _ntff_profile_via_ctypes
        hook = _ntff_profile_via_ctypes("/opt/axon/libaxon_pjrt.so")
        mod = types.ModuleType("antenv.axon_hooks")
        mod.get_axon_ntff_profile_hook = lambda: hook
        mod.set_axon_ntff_profile_hook = lambda h: None
        sys.modules["antenv.axon_hooks"] = mod
        import antenv
        antenv.axon_hooks = mod
    except Exception:
        os.environ["BASS_NEVER_TRACE"] = "1"

# Problem geometry (hardcoded per spec).
B = 64             # queries
D = 768            # feature dim
N = 500000         # feature rows
NCORES = 8
NSH = N // NCORES  # 62500 rows per core
KC = D // 128      # 6 contraction chunks of 128
CHUNK = 500        # matmul moving free dim; PSUM bank holds 512 fp32
NCHUNKS = NSH // CHUNK   # 125
NPAIRS = 62              # pairs (2j, 2j+1) cover chunks 0..123; chunk 124 alone

# DMA groups (chunk counts); contiguous HBM block per group. Fine-grained
# groups keep the matmul/scan pipeline DMA-paced (no bursty group waits);
# small head groups start compute early; small tail groups shrink the
# serial tail after the last byte lands.
GROUPS = [2, 2] + [4] * 29 + [2, 2, 1]
assert sum(GROUPS) == NCHUNKS
CHUNK_ORDER = list(range(NCHUNKS))
CHUNK_POS = {c: p for p, c in enumerate(CHUNK_ORDER)}
GW = max(GROUPS)
PERCH = KC * CHUNK  # bytes per partition per chunk (fp8) = 3000

NBLOCKS = NPAIRS + 1  # one 500-col scan per pair + the lone chunk 124
TOPB = 8
OUTW = NBLOCKS * TOPB  # 504
# Progressive output drains after these block indices (prefix col ranges).
DRAINS = [(30, 0, 31 * TOPB), (55, 31 * TOPB, 56 * TOPB), (NBLOCKS - 1, 56 * TOPB, OUTW)]

_COMPILED = None
LAST_RESULTS = None  # test harness introspection


def _build():
    nc = bacc.Bacc("TRN2", target_bir_lowering=False, debug=False)
    qT = nc.declare_dram_parameter("qT", [128, KC * B], mybir.dt.float8e4, isOutput=False)
    fT = nc.declare_dram_parameter("fT", [NSH * D], mybir.dt.float8e4, isOutput=False)
    out_vals = nc.declare_dram_parameter(
        "vals", [128, OUTW], mybir.dt.float32, isOutput=True
    )
    out_idx = nc.declare_dram_parameter(
        "idx", [128, OUTW], mybir.dt.uint16, isOutput=True
    )

    with TileContext(nc) as tc:
        with (
            tc.tile_pool(name="qpool", bufs=1) as qpool,
            tc.tile_pool(name="fpool", bufs=16) as fpool,
            tc.tile_pool(name="outpool", bufs=1) as outpool,
            tc.tile_pool(name="psum", bufs=8, space="PSUM") as psump,
        ):
            q_sb = qpool.tile([128, KC, B], mybir.dt.float8e4)
            nc.scalar.dma_start(
                out=q_sb[:], in_=qT.ap().rearrange("p (k m) -> p k m", k=KC)
            )

            vals_st = outpool.tile([128, OUTW], mybir.dt.float32)
            idx_st = outpool.tile([128, OUTW], mybir.dt.uint16)

            chunk_views = {}   # chunk id -> SBUF AP [128, KC, CHUNK]
            loaded = [0]
            goff = [0]         # flat fp8 offset of next group
            gidx = [0]

            def load_until(c):
                pos = CHUNK_POS[c]
                while loaded[0] <= pos:
                    gw = GROUPS[gidx[0]]
                    f_sb = fpool.tile([128, GW * PERCH], mybir.dt.float8e4)
                    sz = gw * PERCH
                    nc.sync.dma_start(
                        out=f_sb[:, :sz],
                        in_=fT.ap()[goff[0] : goff[0] + 128 * sz].rearrange(
                            "(p n) -> p n", p=128
                        ),
                    )
                    for ci in range(gw):
                        chunk_views[CHUNK_ORDER[loaded[0] + ci]] = f_sb[
                            :, ci * PERCH : (ci + 1) * PERCH
                        ].rearrange("p (k n) -> p k n", k=KC)
                    goff[0] += 128 * sz
                    loaded[0] += gw
                    gidx[0] += 1

            def mm_half(ps_cols, chunk, half):
                for k in range(KC):
                    nc.tensor.matmul(
                        ps_cols[half * B : (half + 1) * B, :],
                        lhsT=q_sb[:, k, :],
                        rhs=chunk_views[chunk][:, k, :],
                        start=(k == 0),
                        stop=(k == KC - 1),
                        tile_position=(0, half * B) if half else None,
                    )

            for blk in range(NBLOCKS):
                ps = psump.tile([128, 512], mybir.dt.float32)
                if blk < NPAIRS:
                    load_until(2 * blk)
                    mm_half(ps[:, :CHUNK], 2 * blk, 0)
                    load_until(2 * blk + 1)
                    mm_half(ps[:, :CHUNK], 2 * blk + 1, 1)
                else:  # lone chunk 124: partitions 64-127 scan stale PSUM,
                    # and the host drops those slots (lone block, h==1).
                    load_until(NCHUNKS - 1)
                    mm_half(ps[:, :CHUNK], NCHUNKS - 1, 0)
                scan = ps[:, :CHUNK]
                vslot = vals_st[:, blk * TOPB : (blk + 1) * TOPB]
                nc.vector.max(out=vslot, in_=scan)
                nc.vector.max_index(
                    out=idx_st[:, blk * TOPB : (blk + 1) * TOPB],
                    in_max=vslot,
                    in_values=scan,
                )
                for dblk, c0, c1 in DRAINS:
                    if blk == dblk:
                        # Mid-stream drains must stay OFF the sync ring: the
                        # HWDGE queue is in-order, so a drain gated on DVE
                        # progress would block the feature groups behind it.
                        # Only the final idx drain (no features left) uses
                        # sync, so the two last drains complete in parallel.
                        nc.scalar.dma_start(
                            out=out_vals.ap()[:, c0:c1], in_=vals_st[:, c0:c1]
                        )
                        idx_ring = nc.sync if blk == NBLOCKS - 1 else nc.scalar
                        idx_ring.dma_start(
                            out=out_idx.ap()[:, c0:c1], in_=idx_st[:, c0:c1]
                        )

    nc.compile()
    return nc


def _get_compiled():
    global _COMPILED
    if _COMPILED is None:
        _COMPILED = _build()
    return _COMPILED


def _pretile(f_shard, F8):
    """[62500, 768] fp32 -> flat fp8 buffer in per-group contiguous layout
    following CHUNK_ORDER: group g -> [128 partitions][chunk][KC][500],
    partition-major."""
    f8 = f_shard.astype(F8)
    parts = []
    pos = 0
    for gw in GROUPS:
        ids = CHUNK_ORDER[pos : pos + gw]
        rows = np.concatenate([f8[c * CHUNK : (c + 1) * CHUNK] for c in ids])
        sub = rows.reshape(gw, CHUNK, KC, 128)            # (ci, j, k, p)
        parts.append(np.ascontiguousarray(sub.transpose(3, 0, 2, 1)).reshape(-1))
        pos += gw
    return np.concatenate(parts)


def _candidates(idx_arr, val_arr):
    """Map device outputs (128, 504) to per-query (rows, vals).

    Row q < 64 covers the first chunk of each pair (h=0); row q+64 the
    second (h=1). Block b < 62 is pair b; block 62 is the lone chunk 124
    (valid only for h=0). Returns (B, 2*504); invalid slots get -inf val.
    """
    blk = np.repeat(np.arange(NBLOCKS), TOPB)  # (504,)
    lone = blk == NPAIRS
    rows_out = np.empty((B, 2 * OUTW), dtype=np.int64)
    vals_out = np.empty((B, 2 * OUTW), dtype=np.float64)
    for h in (0, 1):
        i = idx_arr[h * B : (h + 1) * B].astype(np.int64)       # (64, 504)
        v = val_arr[h * B : (h + 1) * B].astype(np.float64)
        feat = np.where(lone, (NCHUNKS - 1) * CHUNK + i, (2 * blk + h) * CHUNK + i)
        if h == 1:  # lone chunk block has no h=1 half
            v = np.where(lone, -np.inf, v)
        rows_out[:, h * OUTW : (h + 1) * OUTW] = feat
        vals_out[:, h * OUTW : (h + 1) * OUTW] = v
    return rows_out, vals_out


def kernel(query_feature, feature, data, k=5, **kwargs):
    global LAST_RESULTS
    q = np.ascontiguousarray(np.asarray(query_feature, dtype=np.float32))
    f = np.asarray(feature, dtype=np.float32)
    data = np.asarray(data)
    k = int(k)
    assert q.shape == (B, D) and f.shape == (N, D)

    nc = _get_compiled()

    F8 = mybir.dt.np(mybir.dt.float8e4)
    # qT[p, k*64+m] = q[m, k*128+p]
    qT = np.ascontiguousarray(
        q.astype(F8).reshape(B, KC, 128).transpose(2, 1, 0)
    ).reshape(128, KC * B)
    in_maps = []
    for i in range(NCORES):
        in_maps.append({"qT": qT, "fT": _pretile(f[i * NSH : (i + 1) * NSH], F8)})

    _ensure_ntff_hook()
    res = run_bass_kernel_spmd(nc, in_maps, core_ids=list(range(NCORES)))
    LAST_RESULTS = res

    all_rows, all_vals = [], []
    for i in range(NCORES):
        rows, vals = _candidates(res.results[i]["idx"], res.results[i]["vals"])
        all_rows.append(i * NSH + rows)
        all_vals.append(vals)
    cand_all = np.concatenate(all_rows, axis=1)  # (B, NCORES*1008)
    vals_all = np.concatenate(all_vals, axis=1)

    # Prefilter by device dot value (fp8 noise sigma ~1.4 on margins ~30
    # sigma): keep the top PREK per query, then rescore those exactly.
    PREK = 96
    pre = np.argpartition(-vals_all, PREK, axis=1)[:, :PREK]
    cand = np.take_along_axis(cand_all, pre, axis=1)  # (B, PREK)

    # Exact fp32 rescore of candidates (same math as the reference).
    qn = q / np.linalg.norm(q, axis=1, keepdims=True)
    fc = f[cand]  # (B, C, D)
    fn = fc / np.linalg.norm(fc, axis=2, keepdims=True)
    sims = np.einsum("bd,bcd->bc", qn, fn)  # fp32

    # Final top-k with jax.lax.top_k tie-breaking (value desc, index asc).
    # Exact fp32 ties inside a block can make Max8/MaxIndex emit duplicate
    # candidates: sort by index, mask duplicate neighbors.
    o = np.argsort(cand, axis=1, kind="stable")
    cand_s = np.take_along_axis(cand, o, axis=1)
    sims_s = np.take_along_axis(sims, o, axis=1)
    dup = np.zeros_like(sims_s, dtype=bool)
    dup[:, 1:] = cand_s[:, 1:] == cand_s[:, :-1]
    sims_s = np.where(dup, -np.inf, sims_s)
    sel = np.argsort(-sims_s, axis=1, kind="stable")[:, :k]
    top_idx = np.take_along_axis(cand_s, sel, axis=1)  # (B, k)

    return data[top_idx]  # (B, k, data_cols), input dtype preserved


# revision 38
# speedup vs baseline: 1.0912x; 1.0594x over previous
"""Distributed brute-force kNN retrieval (cosine similarity) on 8 Trainium2 cores.

Strategy:
  - Shard the feature bank along N across 8 cores (62500 rows each).
  - Host pre-tiles each shard into fp8, grouped so every DMA is one fully
    contiguous HBM block (128 partitions x up-to-48KB per partition).
    Groups are small at the start (so the first matmuls/scans begin early)
    and at the end (so the serial tail after the last DMA is minimal).
  - Each core computes raw dot products q @ f_shard.T with fp8 matmuls
    (fp32 PSUM accumulation). A chunk pair (2j, 2j+1) lands in one PSUM
    bank ([128, 512] tile, 500 used): queries x chunk 2j on partitions
    0-63, queries x chunk 2j+1 on partitions 64-127 via PE column tiling
    (tile_position=(0,64)).
  - DVE Max8/MaxIndex run DIRECTLY on PSUM (no PSUM->SBUF copy), one
    500-col scan per pair; the odd 125th chunk is a final half block.
  - Candidate vals/idx accumulate in SBUF and drain to HBM progressively
    (on both HWDGE rings) so the final output DMA is tiny.
  - Host maps candidates to global rows, rescores them exactly in fp32
    (normalized cosine, same math as the reference), reduces to top-k and
    gathers the data segments.

Safety margin: top-8 of every 500-col block when only the global top-5
is needed makes the device pass insensitive to fp8 rounding (dot-noise
sigma ~1.4 vs. rank margins of tens of sigma); the exact host rescore
then removes all remaining matmul error.
"""

import os
import sys

import numpy as np

import concourse.bacc as bacc
import concourse.mybir as mybir
from concourse.tile import TileContext
from concourse.bass_utils import run_bass_kernel_spmd


def _ensure_ntff_hook():
    """run_bass_kernel_spmd(trace) under axon imports antenv.axon_hooks,
    which this container image lacks. Provide the shim (profiling works) or
    disable tracing so a stray BASS_TRACE env var cannot crash the run."""
    try:
        import antenv.axon_hooks  # noqa: F401
        return
    except ImportError:
        pass
    try:
        import types
        from trn_agent_boot.trn_boot import _ntff_profile_via_ctypes
        hook = _ntff_profile_via_ctypes("/opt/axon/libaxon_pjrt.so")
        mod = types.ModuleType("antenv.axon_hooks")
        mod.get_axon_ntff_profile_hook = lambda: hook
        mod.set_axon_ntff_profile_hook = lambda h: None
        sys.modules["antenv.axon_hooks"] = mod
        import antenv
        antenv.axon_hooks = mod
    except Exception:
        os.environ["BASS_NEVER_TRACE"] = "1"

# Problem geometry (hardcoded per spec).
B = 64             # queries
D = 768            # feature dim
N = 500000         # feature rows
NCORES = 8
NSH = N // NCORES  # 62500 rows per core
KC = D // 128      # 6 contraction chunks of 128
CHUNK = 500        # matmul moving free dim; PSUM bank holds 512 fp32
NCHUNKS = NSH // CHUNK   # 125
NPAIRS = 62              # pairs (2j, 2j+1) cover chunks 0..123; chunk 124 alone

# DMA groups (chunk counts); contiguous HBM block per group. Fine-grained
# groups keep the matmul/scan pipeline DMA-paced (no bursty group waits);
# small head groups start compute early; small tail groups shrink the
# serial tail after the last byte lands.
GROUPS = [2, 2] + [4] * 29 + [2, 2, 1]
assert sum(GROUPS) == NCHUNKS
CHUNK_ORDER = list(range(NCHUNKS))
CHUNK_POS = {c: p for p, c in enumerate(CHUNK_ORDER)}
GW = max(GROUPS)
PERCH = KC * CHUNK  # bytes per partition per chunk (fp8) = 3000

# Scan blocks: 15 quad-bank scans (pairs 0..59, strided Max8 over 4 PSUM
# banks), two single-pair scans (pairs 60, 61) so the serial tail stays
# short, and the lone chunk 124. kind: "q"=quad, "s"=single, "l"=lone.
NQUAD = 15
BLOCKS = [("q", i) for i in range(NQUAD)] + [("s", 60), ("s", 61), ("l", 0)]
NBLOCKS = len(BLOCKS)  # 18
TOPB = 8
OUTW = NBLOCKS * TOPB  # 144
# Progressive output drains after these block indices (prefix col ranges).
DRAINS = [(7, 0, 8 * TOPB), (13, 8 * TOPB, 14 * TOPB), (NBLOCKS - 1, 14 * TOPB, OUTW)]

_COMPILED = None
LAST_RESULTS = None  # test harness introspection


def _build():
    nc = bacc.Bacc("TRN2", target_bir_lowering=False, debug=False)
    qT = nc.declare_dram_parameter("qT", [128, KC * B], mybir.dt.float8e4, isOutput=False)
    fT = nc.declare_dram_parameter("fT", [NSH * D], mybir.dt.float8e4, isOutput=False)
    out_vals = nc.declare_dram_parameter(
        "vals", [128, OUTW], mybir.dt.float32, isOutput=True
    )
    out_idx = nc.declare_dram_parameter(
        "idx", [128, OUTW], mybir.dt.uint16, isOutput=True
    )

    with TileContext(nc) as tc:
        with (
            tc.tile_pool(name="qpool", bufs=1) as qpool,
            tc.tile_pool(name="fpool", bufs=16) as fpool,
            tc.tile_pool(name="outpool", bufs=1) as outpool,
            tc.tile_pool(name="psum", bufs=2, space="PSUM") as psump,
        ):
            q_sb = qpool.tile([128, KC, B], mybir.dt.float8e4)
            nc.scalar.dma_start(
                out=q_sb[:], in_=qT.ap().rearrange("p (k m) -> p k m", k=KC)
            )

            vals_st = outpool.tile([128, OUTW], mybir.dt.float32)
            idx_st = outpool.tile([128, OUTW], mybir.dt.uint16)

            chunk_views = {}   # chunk id -> SBUF AP [128, KC, CHUNK]
            loaded = [0]
            goff = [0]         # flat fp8 offset of next group
            gidx = [0]

            def load_until(c):
                pos = CHUNK_POS[c]
                while loaded[0] <= pos:
                    gw = GROUPS[gidx[0]]
                    f_sb = fpool.tile([128, GW * PERCH], mybir.dt.float8e4)
                    sz = gw * PERCH
                    nc.sync.dma_start(
                        out=f_sb[:, :sz],
                        in_=fT.ap()[goff[0] : goff[0] + 128 * sz].rearrange(
                            "(p n) -> p n", p=128
                        ),
                    )
                    for ci in range(gw):
                        chunk_views[CHUNK_ORDER[loaded[0] + ci]] = f_sb[
                            :, ci * PERCH : (ci + 1) * PERCH
                        ].rearrange("p (k n) -> p k n", k=KC)
                    goff[0] += 128 * sz
                    loaded[0] += gw
                    gidx[0] += 1

            def mm_half(ps_cols, chunk, half):
                for k in range(KC):
                    nc.tensor.matmul(
                        ps_cols[half * B : (half + 1) * B, :],
                        lhsT=q_sb[:, k, :],
                        rhs=chunk_views[chunk][:, k, :],
                        start=(k == 0),
                        stop=(k == KC - 1),
                        tile_position=(0, half * B) if half else None,
                    )

            def mm_pair(ps_cols, pair):
                load_until(2 * pair)
                mm_half(ps_cols, 2 * pair, 0)
                load_until(2 * pair + 1)
                mm_half(ps_cols, 2 * pair + 1, 1)

            for blk, (kind, arg) in enumerate(BLOCKS):
                ps = psump.tile([128, 2048], mybir.dt.float32)
                if kind == "q":
                    for a in range(4):
                        mm_pair(ps[:, a * 512 : a * 512 + CHUNK], 4 * arg + a)
                    # Max8 skips the 12-col bank gaps via a strided view, so
                    # stale gap values cannot enter the top-8. MaxIndex needs
                    # a 2-D range and so includes the gaps; the host drops
                    # the (measure-zero) case of a needle matching gap trash.
                    scan_v = ps[:, : 4 * 512].rearrange(
                        "p (a c) -> p a c", a=4
                    )[:, :, :CHUNK]
                    scan_i = ps[:, : 3 * 512 + CHUNK]
                elif kind == "s":
                    mm_pair(ps[:, :CHUNK], arg)
                    scan_v = scan_i = ps[:, :CHUNK]
                else:  # lone chunk 124: partitions 64-127 scan stale PSUM,
                    # and the host drops those slots (lone block, h==1).
                    load_until(NCHUNKS - 1)
                    mm_half(ps[:, :CHUNK], NCHUNKS - 1, 0)
                    scan_v = scan_i = ps[:, :CHUNK]
                vslot = vals_st[:, blk * TOPB : (blk + 1) * TOPB]
                nc.vector.max(out=vslot, in_=scan_v)
                nc.vector.max_index(
                    out=idx_st[:, blk * TOPB : (blk + 1) * TOPB],
                    in_max=vslot,
                    in_values=scan_i,
                )
                for dblk, c0, c1 in DRAINS:
                    if blk == dblk:
                        # Mid-stream drains must stay OFF the sync ring: the
                        # HWDGE queue is in-order, so a drain gated on DVE
                        # progress would block the feature groups behind it.
                        # Only the final idx drain (no features left) uses
                        # sync, so the two last drains complete in parallel.
                        nc.scalar.dma_start(
                            out=out_vals.ap()[:, c0:c1], in_=vals_st[:, c0:c1]
                        )
                        idx_ring = nc.sync if blk == NBLOCKS - 1 else nc.scalar
                        idx_ring.dma_start(
                            out=out_idx.ap()[:, c0:c1], in_=idx_st[:, c0:c1]
                        )

    nc.compile()
    return nc


def _get_compiled():
    global _COMPILED
    if _COMPILED is None:
        _COMPILED = _build()
    return _COMPILED


def _pretile(f_shard, F8):
    """[62500, 768] fp32 -> flat fp8 buffer in per-group contiguous layout
    following CHUNK_ORDER: group g -> [128 partitions][chunk][KC][500],
    partition-major."""
    f8 = f_shard.astype(F8)
    parts = []
    pos = 0
    for gw in GROUPS:
        ids = CHUNK_ORDER[pos : pos + gw]
        rows = np.concatenate([f8[c * CHUNK : (c + 1) * CHUNK] for c in ids])
        sub = rows.reshape(gw, CHUNK, KC, 128)            # (ci, j, k, p)
        parts.append(np.ascontiguousarray(sub.transpose(3, 0, 2, 1)).reshape(-1))
        pos += gw
    return np.concatenate(parts)


# Per-column decode tables (144 cols): block kind and argument.
_KINDQ = np.repeat(np.array([1 if k == "q" else 0 for k, _ in BLOCKS]), TOPB)
_KINDL = np.repeat(np.array([1 if k == "l" else 0 for k, _ in BLOCKS]), TOPB)
_ARG = np.repeat(np.array([a for _, a in BLOCKS]), TOPB)


def _candidates(idx_arr, val_arr):
    """Map device outputs (128, 144) to per-query (rows, vals).

    Row q < 64 covers the first chunk of each pair (h=0); row q+64 the
    second (h=1). Quad block a covers pairs 4a..4a+3 with MaxIndex stream
    position i -> pair offset i//512, column i%512 (columns >= 500 are
    PSUM bank-gap hits, dropped). Single blocks are one pair; the lone
    block is chunk 124 (valid only for h=0). Invalid slots get -inf val.
    """
    rows_out = np.empty((B, 2 * OUTW), dtype=np.int64)
    vals_out = np.empty((B, 2 * OUTW), dtype=np.float64)
    for h in (0, 1):
        i = idx_arr[h * B : (h + 1) * B].astype(np.int64)       # (64, 144)
        v = val_arr[h * B : (h + 1) * B].astype(np.float64)
        pair_q = 4 * _ARG + i // 512
        col_q = i % 512
        quad = (2 * pair_q + h) * CHUNK + np.minimum(col_q, CHUNK - 1)
        single = (2 * _ARG + h) * CHUNK + i
        lone = (NCHUNKS - 1) * CHUNK + np.minimum(i, CHUNK - 1)
        feat = np.where(_KINDQ == 1, quad, np.where(_KINDL == 1, lone, single))
        v = np.where((_KINDQ == 1) & (col_q >= CHUNK), -np.inf, v)
        if h == 1:  # lone chunk block has no h=1 half
            v = np.where(_KINDL == 1, -np.inf, v)
        rows_out[:, h * OUTW : (h + 1) * OUTW] = feat
        vals_out[:, h * OUTW : (h + 1) * OUTW] = v
    return rows_out, vals_out


def kernel(query_feature, feature, data, k=5, **kwargs):
    global LAST_RESULTS
    q = np.ascontiguousarray(np.asarray(query_feature, dtype=np.float32))
    f = np.asarray(feature, dtype=np.float32)
    data = np.asarray(data)
    k = int(k)
    assert q.shape == (B, D) and f.shape == (N, D)

    nc = _get_compiled()

    F8 = mybir.dt.np(mybir.dt.float8e4)
    # qT[p, k*64+m] = q[m, k*128+p]
    qT = np.ascontiguousarray(
        q.astype(F8).reshape(B, KC, 128).transpose(2, 1, 0)
    ).reshape(128, KC * B)
    in_maps = []
    for i in range(NCORES):
        in_maps.append({"qT": qT, "fT": _pretile(f[i * NSH : (i + 1) * NSH], F8)})

    _ensure_ntff_hook()
    res = run_bass_kernel_spmd(nc, in_maps, core_ids=list(range(NCORES)))
    LAST_RESULTS = res

    all_rows, all_vals = [], []
    for i in range(NCORES):
        rows, vals = _candidates(res.results[i]["idx"], res.results[i]["vals"])
        all_rows.append(i * NSH + rows)
        all_vals.append(vals)
    cand_all = np.concatenate(all_rows, axis=1)  # (B, NCORES*1008)
    vals_all = np.concatenate(all_vals, axis=1)

    # Prefilter by device dot value (fp8 noise sigma ~1.4 on margins ~30
    # sigma): keep the top PREK per query, then rescore those exactly.
    PREK = 96
    pre = np.argpartition(-vals_all, PREK, axis=1)[:, :PREK]
    cand = np.take_along_axis(cand_all, pre, axis=1)  # (B, PREK)

    # Exact fp32 rescore of candidates (same math as the reference).
    qn = q / np.linalg.norm(q, axis=1, keepdims=True)
    fc = f[cand]  # (B, C, D)
    fn = fc / np.linalg.norm(fc, axis=2, keepdims=True)
    sims = np.einsum("bd,bcd->bc", qn, fn)  # fp32

    # Final top-k with jax.lax.top_k tie-breaking (value desc, index asc).
    # Exact fp32 ties inside a block can make Max8/MaxIndex emit duplicate
    # candidates: sort by index, mask duplicate neighbors.
    o = np.argsort(cand, axis=1, kind="stable")
    cand_s = np.take_along_axis(cand, o, axis=1)
    sims_s = np.take_along_axis(sims, o, axis=1)
    dup = np.zeros_like(sims_s, dtype=bool)
    dup[:, 1:] = cand_s[:, 1:] == cand_s[:, :-1]
    sims_s = np.where(dup, -np.inf, sims_s)
    sel = np.argsort(-sims_s, axis=1, kind="stable")[:, :k]
    top_idx = np.take_along_axis(cand_s, sel, axis=1)  # (B, k)

    return data[top_idx]  # (B, k, data_cols), input dtype preserved
